# revision 1
# baseline (speedup 1.0000x reference)
"""Trainium2 Bass kernel for a quantized (FP4 e2m1, group-64 scales) MoE layer.

Problem shape (hardcoded): T=2048 tokens, K=2048 hidden, I=1024 intermediate,
E=8 routed experts (top-2), plus an always-on shared expert.

Strategy (8 NeuronCores):
  * Expert-parallel: core e owns routed expert e. The token->expert all-to-all
    is done host-side: for each expert we gather the tokens routed to it
    (merged top-2 slots, capacity C=512) and ship x^T [K, C] in bf16.
  * FP4 handling: the host unpacks the 4-bit fields to fp8_e4m3 (holding
    exactly 2*fp4_value - all exact in e4m3); the device applies the group
    scales (x0.5 folded in) with one tensor_tensor multiply per element
    (split across VectorE and GpSimdE) into SBUF-resident bf16 weights, then
    runs bf16 matmuls with fp32 PSUM accumulation.
  * Permuted contraction orderings: rows of the gate_up operands use
    k' = (c,p) -> k = (p%32)*64 + 4c + p//32 so that every 128-row chunk
    needs scale rows p%32 - one constant [128, N] scale tile serves all
    chunks (no 64x scale replication). Same idea for the down contraction:
    i' = 128c + p -> i = 8p + c, realized on the gate_up side by
    single-stride stationary-operand column APs (step 8, offset c), so
    activations emerge already i'-ordered and the down scale tile is also
    chunk-invariant (lane p -> scale row p//8).
  * Shared expert: token-split, 256 tokens per core; weights streamed through
    the same SBUF pools after the routed phases release them.
  * DMAs are batched into multi-chunk transfers (per-DMA fixed cost ~2us).
  * Combine (scatter-add by routing weights + shared add) on host.
"""

import numpy as np
import ml_dtypes

import concourse.bacc as bacc
import concourse.bass as bass
import concourse.mybir as mybir
import concourse.tile as tile
from concourse import bass_utils, library_config

F32 = mybir.dt.float32
BF16 = mybir.dt.bfloat16
FP8 = mybir.dt.float8e4

NP_BF16 = ml_dtypes.bfloat16
NP_FP8 = ml_dtypes.float8_e4m3

T, K, I, E, TOPK, GS = 2048, 2048, 1024, 8, 2, 64
N_CORES = 8
C = 512            # routed token capacity per expert (max merged load is 511
                   # for the fixed seed; host fallback handles any overflow)
TS = T // N_CORES  # shared-expert tokens per core = 256

KC = K // 128      # 16 contraction chunks for gate_up
IC = I // 128      # 8 contraction chunks for down
KS = K // 512      # 4 output column slices

# 2 * fp4_e2m1 value per nibble (sign bit 3): exact in fp8_e4m3 / bf16.
FP4_2T = np.array(
    [0, 1, 2, 3, 4, 6, 8, 12, 0, -1, -2, -3, -4, -6, -8, -12], dtype=np.float32
)

# Contraction permutations (see module docstring).
_kp = np.arange(K)
KPERM = (_kp % 128 % 32) * 64 + 4 * (_kp // 128) + (_kp % 128) // 32
_ip = np.arange(I)
IPERM = 8 * (_ip % 128) + (_ip // 128)

_GU_LANES = (np.arange(128) % 32)
_D_LANES = (np.arange(128) // 8)

_COMPILED = {}


def _decode_fp8_pairs(packed: np.ndarray, perm: np.ndarray) -> np.ndarray:
    """[R, N] int32 -> fp8 of 2*val, rows permuted, packed as chunk pairs
    [R*8//256, 128, 2N]."""
    shifts = (np.arange(8, dtype=np.int32) * 4)[None, :, None]
    nib = (packed[:, None, :] >> shifts) & 0xF
    vals = FP4_2T[nib].reshape(packed.shape[0] * 8, packed.shape[1])[perm]
    R, N = vals.shape
    out = vals.reshape(R // 256, 2, 128, N).transpose(0, 2, 1, 3)
    return np.ascontiguousarray(out.reshape(R // 256, 128, 2 * N)).astype(NP_FP8)


def _quad_chunks(mat: np.ndarray) -> np.ndarray:
    """[R, N] -> [R//512, 128, 4N] (4 row-chunks side by side)."""
    R, N = mat.shape
    out = mat.reshape(R // 512, 4, 128, N).transpose(0, 2, 1, 3)
    return np.ascontiguousarray(out.reshape(R // 512, 128, 4 * N))


def _scale128(scales: np.ndarray, lane_map: np.ndarray) -> np.ndarray:
    return (scales.astype(np.float32)[lane_map] * 0.5).astype(NP_BF16)


def _build_program(reps=1):
    """Build + compile the SPMD Bass program (identical on every core).
    reps>1 repeats the whole body (for timing-slope measurements)."""
    nc = bacc.Bacc("TRN2", target_bir_lowering=False, debug=False,
                   num_devices=N_CORES)

    # ---- DRAM I/O ----
    xT = nc.dram_tensor("xT", [KC // 4, 128, 4 * C], BF16, kind="ExternalInput")
    probs = nc.dram_tensor("probs", [128, C // 128], F32, kind="ExternalInput")
    v_gu = nc.dram_tensor("v_gu", [KC // 2, 128, 2 * 2 * I], FP8,
                          kind="ExternalInput")
    v_d = nc.dram_tensor("v_d", [IC // 2, 128, 2 * K], FP8,
                         kind="ExternalInput")
    s_gu = nc.dram_tensor("s_gu", [128, 2 * I], BF16, kind="ExternalInput")
    s_rest = nc.dram_tensor("s_rest", [128, 3 * 2048], BF16,
                            kind="ExternalInput")
    xsT = nc.dram_tensor("xsT", [KC // 4, 128, 4 * TS], BF16,
                         kind="ExternalInput")
    vs_gu = nc.dram_tensor("vs_gu", [KC // 2, 128, 2 * 2 * I], FP8,
                           kind="ExternalInput")
    vs_d = nc.dram_tensor("vs_d", [IC // 2, 128, 2 * K], FP8,
                          kind="ExternalInput")
    y = nc.dram_tensor("y", [C, K], F32, kind="ExternalOutput")
    ysh = nc.dram_tensor("ysh", [TS, K], F32, kind="ExternalOutput")

    with tile.TileContext(nc) as tc:
        with (
            tc.tile_pool(name="wgu", bufs=KC + 4) as wgu_pool,
            tc.tile_pool(name="wd", bufs=IC + 2) as wd_pool,
            tc.tile_pool(name="xt", bufs=KC // 4) as xt_pool,
            tc.tile_pool(name="xst", bufs=KC // 4) as xst_pool,
            tc.tile_pool(name="act", bufs=IC) as act_pool,
            tc.tile_pool(name="vq", bufs=3) as vq_pool,
            tc.tile_pool(name="vqp", bufs=3) as vqp_pool,
            tc.tile_pool(name="scl", bufs=1) as scl_pool,
            tc.tile_pool(name="ysb", bufs=2) as ysb_pool,
            tc.tile_pool(name="pr", bufs=1) as pr_pool,
            tc.tile_pool(name="silu", bufs=2) as silu_pool,
            tc.tile_pool(name="ps", bufs=8, space="PSUM") as psum_pool,
        ):
            # load the GPSIMD library up front - the auto-inserted reload
            # would otherwise be isolation-scheduled after DVE quiesces
            nc.gpsimd.load_library(library_config.standard)

            for _rep in range(reps):
                # ---- constant scale tiles (gate_up scales first: they gate the
                # first dequant; the rest is deferred below the hot loads) ----
                sgu_t = scl_pool.tile([128, 2 * I], BF16, tag="scl1")
                nc.scalar.dma_start(sgu_t[:, 0:I], s_gu[:, 0:I])
                nc.scalar.dma_start(sgu_t[:, I:2 * I], s_gu[:, I:2 * I])

                def chain_stages(stages):
                    # keep per-engine dequant queues in stage order; the
                    # scheduler otherwise reorders them by heap priority
                    last = {}
                    for tts in stages:
                        first_of, last_of = {}, {}
                        for eng, ti in tts:
                            first_of.setdefault(id(eng), ti)
                            last_of[id(eng)] = ti
                        for k, ti in first_of.items():
                            if k in last:
                                # ti depends on last[k] (runs after it)
                                tile.add_dep_helper(ti.ins, last[k].ins,
                                                    sync=False,
                                                    reason="dequant stage order")
                        last.update(last_of)

                def dequant_matrix(v_dram, npairs, scale_ap, pool, tag, ncols,
                                   engine_of, split_first=False, dma_order=None,
                                   pool_pairs=()):
                    vts = {}
                    tt_insts = []
                    for j in dma_order or range(npairs):
                        if j in pool_pairs:
                            vt = vqp_pool.tile([128, 2 * ncols], FP8, tag="vqp")
                        else:
                            vt = vq_pool.tile([128, 2 * ncols], FP8, tag="vq")
                        nsub = 4 if (split_first and j == 0) else 1
                        sub = 2 * ncols // nsub
                        for u in range(nsub):
                            nc.sync.dma_start(vt[:, u * sub:(u + 1) * sub],
                                              v_dram[j, :, u * sub:(u + 1) * sub])
                        vts[j] = vt
                    tiles = []
                    for ch in range(2 * npairs):
                        j, h = ch // 2, ch % 2
                        vt = vts[j]
                        wt = pool.tile([128, ncols], BF16, tag=tag)
                        eng = engine_of(ch)
                        if split_first and j == 0:  # halve the startup dep chain
                            for u in range(2):
                                ti = eng.tensor_tensor(
                                    wt[:, u * ncols // 2:(u + 1) * ncols // 2],
                                    vt[:, (2 * h + u) * ncols // 2:
                                          (2 * h + u + 1) * ncols // 2],
                                    scale_ap[:, u * ncols // 2:
                                             (u + 1) * ncols // 2],
                                    mybir.AluOpType.mult)
                        else:
                            ti = eng.tensor_tensor(
                                wt[:], vt[:, h * ncols:(h + 1) * ncols],
                                scale_ap, mybir.AluOpType.mult)
                        tiles.append(wt)
                        tt_insts.append((eng, ti))
                    return tiles, tt_insts

                def mlp(wgu_tiles, wd_tiles, xt_of, tcnt, y_dram, pr_ap):
                    """gate_up matmul + silu*up + down matmul + combine-scale."""
                    tchunks = tcnt // 128
                    # -- gate_up: for each down-chunk c, produce act'[c] [128, t]
                    # directly in i'-row order via strided stationary columns.
                    act_tiles = []
                    for c in range(IC):
                        hpair = []
                        for half in range(2):     # 0: gate, 1: up
                            ps = psum_pool.tile([128, tcnt], F32, tag="ps")
                            for k in range(KC):
                                lhs = (wgu_tiles[k][:, half * I:(half + 1) * I]
                                       .rearrange("p (r g) -> p g r",
                                                  r=128, g=8)[:, c, :])
                                nc.tensor.matmul(
                                    ps[:], lhs, xt_of(k),
                                    start=(k == 0), stop=(k == KC - 1),
                                )
                            hpair.append(ps)
                        gate_ps, up_ps = hpair
                        sil = silu_pool.tile([128, tcnt], BF16, tag="silu")
                        nc.scalar.activation(sil[:], gate_ps[:],
                                             mybir.ActivationFunctionType.Silu)
                        at = act_pool.tile([128, tcnt], BF16, tag="act")
                        nc.vector.tensor_tensor(at[:], sil[:], up_ps[:],
                                                mybir.AluOpType.mult)
                        act_tiles.append(at)

                    # -- down: y[t, k] = act'[i', t].T @ Wd'[i', k], x probs
                    for tb in range(tchunks):
                        last_tb = tb == tchunks - 1
                        for kh in range(2):
                            ot = ysb_pool.tile([128, K // 2], F32, tag="ysb")
                            for ks in (2 * kh, 2 * kh + 1):
                                ps = psum_pool.tile([128, 512], F32, tag="ps")
                                for c in range(IC):
                                    nc.tensor.matmul(
                                        ps[:],
                                        act_tiles[c][:, tb * 128:(tb + 1) * 128],
                                        wd_tiles[c][:, ks * 512:(ks + 1) * 512],
                                        start=(c == 0), stop=(c == IC - 1),
                                    )
                                osl = ot[:, (ks % 2) * 512:(ks % 2 + 1) * 512]
                                if pr_ap is None:
                                    if last_tb and ks >= KS - 2:
                                        # final copies split ACT/DVE, small
                                        # pieces -> short kernel tail
                                        for u in range(2):
                                            sl = osl[:, u * 256:(u + 1) * 256]
                                            pp = ps[:, u * 256:(u + 1) * 256]
                                            if u == 0:
                                                nc.scalar.copy(sl, pp)
                                            else:
                                                nc.vector.tensor_copy(sl, pp)
                                    else:
                                        nc.scalar.copy(osl, ps[:])
                                else:
                                    nc.scalar.activation(
                                        osl, ps[:],
                                        mybir.ActivationFunctionType.Copy,
                                        scale=pr_ap[:, tb:tb + 1])
                                if last_tb:   # shorten the kernel tail
                                    if pr_ap is None and ks == KS - 1:
                                        nc.sync.dma_start(
                                            y_dram[tb * 128:(tb + 1) * 128,
                                                   ks * 512:ks * 512 + 256],
                                            osl[:, 0:256])
                                        nc.scalar.dma_start(
                                            y_dram[tb * 128:(tb + 1) * 128,
                                                   ks * 512 + 256:(ks + 1) * 512],
                                            osl[:, 256:512])
                                    else:
                                        nc.sync.dma_start(
                                            y_dram[tb * 128:(tb + 1) * 128,
                                                   ks * 512:(ks + 1) * 512], osl)
                            if not last_tb:
                                nc.sync.dma_start(
                                    y_dram[tb * 128:(tb + 1) * 128,
                                           kh * 1024:(kh + 1) * 1024], ot[:])

                # ---- routed expert ----
                xt_tiles = []
                for q in range(KC // 4):
                    xt_t = xt_pool.tile([128, 4 * C], BF16, tag="xt")
                    nc.scalar.dma_start(xt_t[:], xT[q, :, :])
                    xt_tiles.append(xt_t)

                def xt_of(k):
                    return xt_tiles[k // 4][:, (k % 4) * C:(k % 4 + 1) * C]

                wgu_tiles, gu_tts = dequant_matrix(
                    v_gu, KC // 2, sgu_t[:], wgu_pool, "wgu", 2 * I,
                    lambda i: nc.vector if i < 10 else nc.gpsimd,
                    split_first=True, dma_order=[5, 0, 1, 2, 3, 6, 4, 7],
                    pool_pairs=(5, 6, 7))

                srest_t = scl_pool.tile([128, 3 * 2048], BF16, tag="scl2")
                nc.sync.dma_start(srest_t[:], s_rest[:, :])
                sd_t = srest_t[:, 0:2048]
                ssgu_t = srest_t[:, 2048:4096]
                ssd_t = srest_t[:, 4096:6144]
                pr_t = pr_pool.tile([128, C // 128], F32, tag="pr")
                nc.sync.dma_start(pr_t[:], probs[:, :])

                wd_tiles, wd_tts = dequant_matrix(
                    v_d, IC // 2, sd_t, wd_pool, "wd", K,
                    lambda i: nc.gpsimd if i < 4 else nc.vector,
                    pool_pairs=(0, 1))

                xst_tiles = []
                for q in range(KC // 4):
                    xs_t = xst_pool.tile([128, 4 * TS], BF16, tag="xst")
                    nc.sync.dma_start(xs_t[:], xsT[q, :, :])
                    xst_tiles.append(xs_t)

                def xst_of(k):
                    return xst_tiles[k // 4][:, (k % 4) * TS:(k % 4 + 1) * TS]

                mlp(wgu_tiles, wd_tiles, xt_of, C, y, pr_t)

                # ---- shared expert (reuses the weight pools' SBUF) ----

                wsgu_tiles, wsgu_tts = dequant_matrix(
                    vs_gu, KC // 2, ssgu_t, wgu_pool, "wgu", 2 * I,
                    lambda i: nc.vector if i < 10 else nc.gpsimd,
                    pool_pairs=(5, 6, 7))
                wsd_tiles, wsd_tts = dequant_matrix(
                    vs_d, IC // 2, ssd_t, wd_pool, "wd", K,
                    lambda i: nc.vector if i < 6 else nc.gpsimd,
                    pool_pairs=(3,))
                chain_stages([gu_tts, wd_tts, wsgu_tts, wsd_tts])

                mlp(wsgu_tiles, wsd_tiles, xst_of, TS, ysh, None)

    nc.compile()
    return nc


def _get_program():
    if "nc" not in _COMPILED:
        _COMPILED["nc"] = _build_program()
    return _COMPILED["nc"]


def kernel(**inputs) -> np.ndarray:
    x = np.asarray(inputs["hidden_states"], np.float32)          # [T, K]
    gu_p = np.asarray(inputs["gate_up_weight_packed"])           # [E, K/8, 2I]
    gu_s = np.asarray(inputs["gate_up_scales"], np.float32)      # [E, K/GS, 2I]
    d_p = np.asarray(inputs["down_weight_packed"])               # [E, I/8, K]
    d_s = np.asarray(inputs["down_scales"], np.float32)          # [E, I/GS, K]
    sgu_p = np.asarray(inputs["shared_gate_up_packed"])          # [K/8, 2I]
    sgu_s = np.asarray(inputs["shared_gate_up_scales"], np.float32)
    sd_p = np.asarray(inputs["shared_down_packed"])              # [I/8, K]
    sd_s = np.asarray(inputs["shared_down_scales"], np.float32)
    eids = np.asarray(inputs["expert_ids"])                      # [T, TOPK]
    eprobs = np.asarray(inputs["expert_probs"], np.float32)      # [T, TOPK]

    # ---- host routing: merged combine weights, token gather per expert ----
    combine = np.zeros((T, E), np.float32)
    np.add.at(combine, (np.arange(T)[:, None], eids), eprobs)
    idx_list = [np.nonzero(combine[:, e])[0] for e in range(E)]
    overflow = max(len(i) for i in idx_list) > C

    xbf = x.astype(NP_BF16)
    xbf_perm_T = np.ascontiguousarray(xbf.T[KPERM])              # [K, T]
    shared_vgu = _decode_fp8_pairs(sgu_p, KPERM)
    shared_vd = _decode_fp8_pairs(sd_p, IPERM)

    in_maps = []
    for e in range(E):
        idx = idx_list[e][:C]
        xT_e = np.zeros((K, C), NP_BF16)
        xT_e[:, :len(idx)] = xbf_perm_T[:, idx]
        pr_full = np.zeros(C, np.float32)
        pr_full[:len(idx)] = combine[idx, e]
        pr_e = np.ascontiguousarray(pr_full.reshape(C // 128, 128).T)
        s_rest_e = np.concatenate(
            [_scale128(d_s[e], _D_LANES),
             _scale128(sgu_s, _GU_LANES),
             _scale128(sd_s, _D_LANES)], axis=1)
        in_maps.append({
            "xT": _quad_chunks(xT_e),
            "probs": pr_e,
            "v_gu": _decode_fp8_pairs(gu_p[e], KPERM),
            "s_gu": _scale128(gu_s[e], _GU_LANES),
            "v_d": _decode_fp8_pairs(d_p[e], IPERM),
            "s_rest": np.ascontiguousarray(s_rest_e),
            "xsT": _quad_chunks(
                np.ascontiguousarray(xbf_perm_T[:, e * TS:(e + 1) * TS])),
            "vs_gu": shared_vgu,
            "vs_d": shared_vd,
        })

    nc = _get_program()
    res = bass_utils.run_bass_kernel_spmd(nc, in_maps,
                                          core_ids=list(range(N_CORES)))

    # ---- host combine ----
    out = np.zeros((T, K), np.float32)
    for e in range(E):
        idx = idx_list[e][:C]
        out[idx] += res.results[e]["y"][:len(idx)]
        out[e * TS:(e + 1) * TS] += res.results[e]["ysh"]

    if overflow:
        # pathological load imbalance: finish dropped tokens on host (exact)
        for e in range(E):
            extra = idx_list[e][C:]
            if len(extra) == 0:
                continue
            wgu = _dequant_full(gu_p[e], gu_s[e])
            wd = _dequant_full(d_p[e], d_s[e])
            h = x[extra] @ wgu
            g, u = h[:, :I], h[:, I:]
            a = (g / (1 + np.exp(-g))) * u
            out[extra] += (a @ wd) * combine[extra, e][:, None]
    return out


def _dequant_full(packed, scales):
    shifts = (np.arange(8, dtype=np.int32) * 4)[None, :, None]
    nib = (packed[:, None, :] >> shifts) & 0xF
    w = FP4_2T[nib].reshape(packed.shape[0] * 8, packed.shape[1]) * 0.5
    return w * np.repeat(scales.astype(np.float32), GS, axis=0)



# revision 2
# speedup vs baseline: 1.0202x; 1.0202x over previous
"""Trainium2 Bass kernel for the FP4-quantized MoE layer — fp8 DoubleRow edition.

Problem (hardcoded): T=2048 tokens, K=2048 hidden, I=1024 intermediate,
E=8 routed experts (top-2, merged), plus an always-on shared expert.

Strategy (8 NeuronCores, expert-parallel; core e owns routed expert e and a
256-token slice of the shared expert):

  * All matmuls run as fp8(e4m3) MatmulPerfMode.DoubleRow: each instruction
    contracts 2x128 rows at 0.5 cycles per output column.
  * Precision: every operand is a 2-plane fp8 decomposition (hi = fp8(v),
    lo = fp8(v - hi)); products accumulate hi*hi + lo*hi + hi*lo in one PSUM
    group (the lo*lo term is dropped; ~1e-3 relative).  Weights are
    pre-scaled by 32 on the host so they sit in e4m3's normal range
    (avoiding the subnormal swamp below 2^-6); activations are re-scaled to
    a 2x domain on the device (max |2*act| ~ 129 < 240).  The scale factors
    cancel via the silu input scale (1/32) and the final copy scale
    (probs/64), both free.
  * Weight planes are decoded+scaled host-side and shipped as fp8 chunk-pair
    tiles; no on-device dequant at all.
  * Outputs ship as fp16 (0.05% of the error budget); host does the
    scatter-add combine.
"""

import numpy as np
import ml_dtypes

import concourse.bacc as bacc
import concourse.bass as bass
import concourse.mybir as mybir
import concourse.tile as tile
from concourse import bass_utils, library_config

F32 = mybir.dt.float32
F16 = mybir.dt.float16
BF16 = mybir.dt.bfloat16
FP8 = mybir.dt.float8e4

NP_F8 = ml_dtypes.float8_e4m3
NP_BF = ml_dtypes.bfloat16

T, K, I, E, TOPK, GS = 2048, 2048, 1024, 8, 2, 64
N_CORES = 8
C = 512            # routed token capacity per expert
TS = T // N_CORES  # shared-expert tokens per core

KP = K // 256      # gate_up contraction chunk-pairs (8)
IP = I // 256      # down contraction chunk-pairs (4)
SW = 32.0          # weight plane domain scale
KA = 2.0           # act plane domain scale
DR = mybir.MatmulPerfMode.DoubleRow

FP4_T = np.array([0, .5, 1, 1.5, 2, 3, 4, 6, 0, -.5, -1, -1.5, -2, -3, -4, -6],
                 dtype=np.float32)

_COMPILED = {}


def _build_program(reps=1):
    nc = bacc.Bacc("TRN2", target_bir_lowering=False, debug=False,
                   num_devices=N_CORES)

    # ---- DRAM I/O (all fp8 planes host-prepared; layouts match SBUF) ----
    # x planes per pair j: [128, 2, tcnt]  (row p -> k = 256j + 128c + p)
    d_xh = nc.dram_tensor("xh", [KP, 128, 2 * C], FP8, kind="ExternalInput")
    d_xl = nc.dram_tensor("xl", [KP, 128, 2 * C], FP8, kind="ExternalInput")
    d_xs = nc.dram_tensor("xs", [KP, 128, 4 * TS], FP8, kind="ExternalInput")
    # gate_up weight planes: per pair j: [128, 2, 2I]  (cols = c*2I + n)
    d_wguh = nc.dram_tensor("wguh", [KP, 128, 2 * 2 * I], FP8,
                            kind="ExternalInput")
    d_wgul = nc.dram_tensor("wgul", [KP, 128, 2 * 2 * I], FP8,
                            kind="ExternalInput")
    d_wdh = nc.dram_tensor("wdh", [IP, 128, 2 * K], FP8, kind="ExternalInput")
    d_wdl = nc.dram_tensor("wdl", [IP, 128, 2 * K], FP8, kind="ExternalInput")
    d_wsguh = nc.dram_tensor("wsguh", [KP, 128, 2 * 2 * I], FP8,
                             kind="ExternalInput")
    d_wsgul = nc.dram_tensor("wsgul", [KP, 128, 2 * 2 * I], FP8,
                             kind="ExternalInput")
    d_wsdh = nc.dram_tensor("wsdh", [IP, 128, 2 * K], FP8, kind="ExternalInput")
    d_wsdl = nc.dram_tensor("wsdl", [IP, 128, 2 * K], FP8, kind="ExternalInput")
    d_pr = nc.dram_tensor("pr", [128, C // 128], F32, kind="ExternalInput")
    d_y = nc.dram_tensor("y", [C, K], F16, kind="ExternalOutput")
    d_ysh = nc.dram_tensor("ysh", [TS, K], F16, kind="ExternalOutput")

    with tile.TileContext(nc) as tc:
        with (
            tc.tile_pool(name="wgu", bufs=3 * KP - 2) as wgu_pool,
            tc.tile_pool(name="wd", bufs=2 * IP + 2) as wd_pool,
            tc.tile_pool(name="xp", bufs=2 * KP) as x_pool,
            tc.tile_pool(name="xsp", bufs=KP) as xs_pool,
            tc.tile_pool(name="pr", bufs=1) as pr_pool,
            tc.tile_pool(name="sil", bufs=2) as sil_pool,
            tc.tile_pool(name="a32", bufs=2) as a32_pool,
            tc.tile_pool(name="ak", bufs=2) as ak_pool,
            tc.tile_pool(name="ah", bufs=4) as ah_pool,
            tc.tile_pool(name="al", bufs=4) as al_pool,
            tc.tile_pool(name="ysb", bufs=22) as ysb_pool,
            tc.tile_pool(name="ps", bufs=8, space="PSUM") as ps_pool,
        ):
            nc.gpsimd.load_library(library_config.standard)

            for _rep in range(reps):
                # ---- input DMAs, in transfer-priority order (all on SP) ----
                def wload(pool, dram, n, width, tag):
                    ts = []
                    for j in range(n):
                        wt = pool.tile([128, width], FP8, tag=tag, name="wt")
                        nc.sync.dma_start(wt[:], dram[j, :, :])
                        ts.append(wt)
                    return ts

                # cold start: interleave x pairs with gate_up hi pairs so the
                # first PSUM groups can open as soon as possible
                xh_t, wguh_t = [], []
                pr_t = None
                for j in range(KP):
                    xt = x_pool.tile([128, 2 * C], FP8, tag="xh", name="xt")
                    nc.sync.dma_start(xt[:], d_xh[j, :, :])
                    xh_t.append(xt)
                    wt = wgu_pool.tile([128, 2 * 2 * I], FP8, tag="wgu",
                                       name="wt")
                    nc.sync.dma_start(wt[:], d_wguh[j, :, :])
                    wguh_t.append(wt)
                    if j == 0:
                        pr_t = pr_pool.tile([128, C // 128], F32, tag="pr")
                        nc.sync.dma_start(pr_t[:], d_pr[:, :])
                xl_t = wload(x_pool, d_xl, KP, 2 * C, "xl")
                wgul_t = wload(wgu_pool, d_wgul, KP, 2 * 2 * I, "wgu")
                wdh_t = wload(wd_pool, d_wdh, IP, 2 * K, "wd")
                wdl_t = wload(wd_pool, d_wdl, IP, 2 * K, "wd")
                xs_t = wload(xs_pool, d_xs, KP, 4 * TS, "xs")
                wsguh_t = wload(wgu_pool, d_wsguh, KP, 2 * 2 * I, "wgu")
                wsgul_t = wload(wgu_pool, d_wsgul, KP, 2 * 2 * I, "wgu")
                wsdh_t = wload(wd_pool, d_wsdh, IP, 2 * K, "wd")
                wsdl_t = wload(wd_pool, d_wsdl, IP, 2 * K, "wd")

                def gu_phase(whi, wlo, xhs, xls, tcnt, ah_ts, al_ts):
                    """gate_up matmuls (DoubleRow, 3 pass-sets) + act planes.

                    k-pair-outer wave order so the PE consumes each weight
                    pair across every open PSUM group as it arrives from DMA
                    (per-bank order would serialize on pair arrivals).
                    Routed (tcnt=512): gate/up each own a bank; waves of 4
                    chunks.  Shared (tcnt=256): gate|up packed into one bank;
                    a single wave of all 8 chunks.
                    """
                    nbt = tcnt // 256
                    packed = nbt == 1
                    x3h, x3l = [], []
                    for th, tl in zip(xhs, xls):
                        x3h.append(th.rearrange("p (c t) -> p c t", c=2))
                        x3l.append(tl.rearrange("p (c t) -> p c t", c=2))
                    waves = ([list(range(8))] if packed
                             else [[0, 1, 2], [3, 4, 5], [6, 7]])
                    for wave in waves:
                        ps_full = {}
                        ps_of = {}
                        for c in wave:
                            if packed:
                                ps = ps_pool.tile([128, 512], F32, tag="ps",
                                                  name="pst")
                                ps_full[c] = ps
                                ps_of[(c, 0)] = ps[:, 0:256]
                                ps_of[(c, 1)] = ps[:, 256:512]
                            else:
                                for half in range(2):
                                    ps = ps_pool.tile([128, 512], F32,
                                                      tag="ps", name="pst")
                                    ps_full[(c, half)] = ps
                                    ps_of[(c, half)] = ps
                        started = set()
                        passes = ((x3h, whi), (x3l, whi), (x3h, wlo))
                        for pi, (xt3, wts) in enumerate(passes):
                            # first two passes j-outer (stream weight-pair
                            # arrivals across all groups); final pass c-outer
                            # so each chunk's group closes early and its act
                            # chain overlaps the remaining matmuls
                            if pi < 2:
                                order = [(j, c) for j in range(KP)
                                         for c in wave]
                            else:
                                order = [(j, c) for c in wave
                                         for j in range(KP)]
                            for j, c in order:
                                if True:
                                    for half in range(2):
                                        w3 = wts[j].rearrange(
                                            "p (c n) -> p c n", c=2)
                                        lhsT = w3[:, :,
                                                  half * I + c * 128:
                                                  half * I + (c + 1) * 128]
                                        gkey = c if packed else (c, half)
                                        for tb in range(nbt):
                                            rhs = xt3[j][:, :,
                                                         tb * 256:
                                                         (tb + 1) * 256]
                                            out = ps_of[(c, half)]
                                            if not packed:
                                                out = out[:, tb * 256:
                                                          (tb + 1) * 256]
                                            last = (pi == 2 and j == KP - 1
                                                    and half == 1
                                                    and tb == nbt - 1)
                                            nc.tensor.matmul(
                                                out, lhsT, rhs,
                                                start=gkey not in started,
                                                stop=last, perf_mode=DR)
                                            started.add(gkey)
                        for c in wave:
                            if packed:
                                gate_ps = ps_full[c][:, 0:256]
                                up_ps = ps_full[c][:, 256:512]
                            else:
                                gate_ps = ps_full[(c, 0)][:, 0:tcnt]
                                up_ps = ps_full[(c, 1)][:, 0:tcnt]
                            sil = sil_pool.tile([128, 512], BF16, tag="sil",
                                                name="silt")
                            nc.scalar.activation(
                                sil[:, 0:tcnt], gate_ps,
                                mybir.ActivationFunctionType.Silu,
                                scale=1.0 / SW)
                            a32 = a32_pool.tile([128, 512], BF16, tag="a32",
                                                name="a32t")
                            nc.vector.tensor_tensor(
                                a32[:, 0:tcnt], sil[:, 0:tcnt], up_ps,
                                mybir.AluOpType.mult)
                            ak = ak_pool.tile([128, 512], BF16, tag="ak",
                                              name="akt")
                            nc.vector.tensor_scalar_mul(
                                ak[:, 0:tcnt], a32[:, 0:tcnt], KA / SW)
                            p, slot = c // 2, c % 2
                            nc.vector.tensor_copy(
                                ah_ts[p][:, slot * 512:slot * 512 + tcnt],
                                ak[:, 0:tcnt])
                            nc.vector.tensor_tensor(
                                al_ts[p][:, slot * 512:slot * 512 + tcnt],
                                ak[:, 0:tcnt],
                                ah_ts[p][:, slot * 512:slot * 512 + tcnt],
                                mybir.AluOpType.subtract)

                def d_phase(wdhi, wdlo, ah_ts, al_ts, tcnt, y_dram, pr_ap):
                    """down matmuls (DoubleRow, 3 pass-sets) + scaled output."""
                    for tb2 in range(tcnt // 128):
                        for kq in range(K // 512):
                            ps = ps_pool.tile([128, 512], F32, tag="ps", name="pst")
                            first = True
                            for kh in range(2):
                                for ats, wts in ((ah_ts, wdhi), (al_ts, wdhi),
                                                 (ah_ts, wdlo)):
                                    for p in range(IP):
                                        a3 = ats[p].rearrange(
                                            "p (c t) -> p c t", c=2)
                                        lhsT = a3[:, :,
                                                  tb2 * 128:(tb2 + 1) * 128]
                                        w3 = wts[p].rearrange(
                                            "p (c k) -> p c k", c=2)
                                        rhs = w3[:, :,
                                                 kq * 512 + kh * 256:
                                                 kq * 512 + (kh + 1) * 256]
                                        last = (kh == 1 and wts is wdlo
                                                and p == IP - 1)
                                        nc.tensor.matmul(
                                            ps[:, kh * 256:(kh + 1) * 256],
                                            lhsT, rhs,
                                            start=first, stop=last,
                                            perf_mode=DR)
                                        first = False
                            ot = ysb_pool.tile([128, 512], F16, tag="ysb", name="ysbt")
                            last_it = (pr_ap is None
                                       and tb2 == tcnt // 128 - 1
                                       and kq == K // 512 - 1)
                            if last_it:
                                # kernel tail: split copy across ACT/DVE and
                                # the DMA across SWDGE/HWDGE paths
                                nc.scalar.activation(
                                    ot[:, 0:256], ps[:, 0:256],
                                    mybir.ActivationFunctionType.Copy,
                                    scale=1.0 / (SW * KA))
                                nc.vector.tensor_scalar_mul(
                                    ot[:, 256:512], ps[:, 256:512],
                                    1.0 / (SW * KA))
                                nc.sync.dma_start(
                                    y_dram[tb2 * 128:(tb2 + 1) * 128,
                                           kq * 512:kq * 512 + 256],
                                    ot[:, 0:256])
                                nc.scalar.dma_start(
                                    y_dram[tb2 * 128:(tb2 + 1) * 128,
                                           kq * 512 + 256:(kq + 1) * 512],
                                    ot[:, 256:512])
                            elif pr_ap is None:
                                nc.scalar.activation(
                                    ot[:], ps[:],
                                    mybir.ActivationFunctionType.Copy,
                                    scale=1.0 / (SW * KA))
                                eng = (nc.gpsimd, nc.scalar,
                                       nc.sync)[(tb2 * (K // 512) + kq) % 3]
                                eng.dma_start(
                                    y_dram[tb2 * 128:(tb2 + 1) * 128,
                                           kq * 512:(kq + 1) * 512], ot[:])
                            else:
                                nc.scalar.activation(
                                    ot[:], ps[:],
                                    mybir.ActivationFunctionType.Copy,
                                    scale=pr_ap[:, tb2:tb2 + 1])
                                nc.gpsimd.dma_start(
                                    y_dram[tb2 * 128:(tb2 + 1) * 128,
                                           kq * 512:(kq + 1) * 512], ot[:])

                # act plane tiles (pair tiles: [slot0 | slot1], 512 cols each)
                r_ah = [ah_pool.tile([128, 1024], FP8, tag="ah", name="aht")
                        for _ in range(IP)]
                r_al = [al_pool.tile([128, 1024], FP8, tag="al", name="alt")
                        for _ in range(IP)]
                gu_phase(wguh_t, wgul_t, xh_t, xl_t, C, r_ah, r_al)
                d_phase(wdh_t, wdl_t, r_ah, r_al, C, d_y, pr_t)

                s_ah = [ah_pool.tile([128, 1024], FP8, tag="ah", name="aht")
                        for _ in range(IP)]
                s_al = [al_pool.tile([128, 1024], FP8, tag="al", name="alt")
                        for _ in range(IP)]
                gu_phase(wsguh_t, wsgul_t,
                         [t[:, 0:2 * TS] for t in xs_t],
                         [t[:, 2 * TS:4 * TS] for t in xs_t],
                         TS, s_ah, s_al)
                d_phase(wsdh_t, wsdl_t, s_ah, s_al, TS, d_ysh, None)

    nc.compile()
    return nc


def _get_program():
    if "nc" not in _COMPILED:
        _COMPILED["nc"] = _build_program()
    return _COMPILED["nc"]


def _dequant32(packed, scales):
    """fp4-packed [R/8, N] + scales [R/GS, N] -> 32x-scaled fp32 [R, N]."""
    shifts = (np.arange(8, dtype=np.int32) * 4)[None, :, None]
    nib = (packed[:, None, :] >> shifts) & 0xF
    w = FP4_T[nib].reshape(packed.shape[0] * 8, packed.shape[1])
    return (w * np.repeat(scales.astype(np.float32), GS, axis=0)) * SW


def _planes(w32):
    """fp32 (already 32x) -> (hi, lo) fp8 planes."""
    hi = w32.astype(NP_F8)
    lo = (w32 - hi.astype(np.float32)).astype(NP_F8)
    return hi, lo


def _pair_layout_w(plane, npair):
    """[R, N] -> [npair, 128, 2*N]: row r = 256j + 128c + p."""
    R, N = plane.shape
    out = plane.reshape(npair, 2, 128, N).transpose(0, 2, 1, 3)
    return np.ascontiguousarray(out.reshape(npair, 128, 2 * N))


def _pair_layout_x(xT):
    """[K, tcnt] -> [KP, 128, 2*tcnt] chunk-pair layout."""
    Kd, tc = xT.shape
    out = xT.reshape(KP, 2, 128, tc).transpose(0, 2, 1, 3)
    return np.ascontiguousarray(out.reshape(KP, 128, 2 * tc))


def kernel(**inputs) -> np.ndarray:
    x = np.asarray(inputs["hidden_states"], np.float32)          # [T, K]
    gu_p = np.asarray(inputs["gate_up_weight_packed"])           # [E, K/8, 2I]
    gu_s = np.asarray(inputs["gate_up_scales"], np.float32)
    d_p = np.asarray(inputs["down_weight_packed"])               # [E, I/8, K]
    d_s = np.asarray(inputs["down_scales"], np.float32)
    sgu_p = np.asarray(inputs["shared_gate_up_packed"])
    sgu_s = np.asarray(inputs["shared_gate_up_scales"], np.float32)
    sd_p = np.asarray(inputs["shared_down_packed"])
    sd_s = np.asarray(inputs["shared_down_scales"], np.float32)
    eids = np.asarray(inputs["expert_ids"])
    eprobs = np.asarray(inputs["expert_probs"], np.float32)

    # ---- host routing ----
    combine = np.zeros((T, E), np.float32)
    np.add.at(combine, (np.arange(T)[:, None], eids), eprobs)
    idx_list = [np.nonzero(combine[:, e])[0] for e in range(E)]
    overflow = max(len(i) for i in idx_list) > C

    # x planes, full [K, T] once
    xT = np.ascontiguousarray(x.T)
    xh_full = xT.astype(NP_F8)
    xl_full = (xT - xh_full.astype(np.float32)).astype(NP_F8)

    # shared weight planes (identical on every core)
    wsgu_hi, wsgu_lo = _planes(_dequant32(sgu_p, sgu_s))
    wsd_hi, wsd_lo = _planes(_dequant32(sd_p, sd_s))
    shared_w = {
        "wsguh": _pair_layout_w(wsgu_hi, KP),
        "wsgul": _pair_layout_w(wsgu_lo, KP),
        "wsdh": _pair_layout_w(wsd_hi, IP),
        "wsdl": _pair_layout_w(wsd_lo, IP),
    }

    in_maps = []
    for e in range(E):
        idx = idx_list[e][:C]
        ncnt = len(idx)
        xh_e = np.zeros((K, C), NP_F8)
        xl_e = np.zeros((K, C), NP_F8)
        xh_e[:, :ncnt] = xh_full[:, idx]
        xl_e[:, :ncnt] = xl_full[:, idx]
        pr_full = np.zeros(C, np.float32)
        pr_full[:ncnt] = combine[idx, e] / (SW * KA)
        pr_e = np.ascontiguousarray(pr_full.reshape(C // 128, 128).T)

        wgu_hi, wgu_lo = _planes(_dequant32(gu_p[e], gu_s[e]))
        wd_hi, wd_lo = _planes(_dequant32(d_p[e], d_s[e]))
        sl = slice(e * TS, (e + 1) * TS)
        in_maps.append({
            "xh": _pair_layout_x(xh_e),
            "xl": _pair_layout_x(xl_e),
            "xs": np.concatenate([_pair_layout_x(xh_full[:, sl]),
                                  _pair_layout_x(xl_full[:, sl])], axis=2),
            "wguh": _pair_layout_w(wgu_hi, KP),
            "wgul": _pair_layout_w(wgu_lo, KP),
            "wdh": _pair_layout_w(wd_hi, IP),
            "wdl": _pair_layout_w(wd_lo, IP),
            "pr": pr_e,
            **shared_w,
        })

    nc = _get_program()
    res = bass_utils.run_bass_kernel_spmd(nc, in_maps,
                                          core_ids=list(range(N_CORES)))

    # ---- host combine ----
    out = np.zeros((T, K), np.float32)
    for e in range(E):
        idx = idx_list[e][:C]
        out[idx] += res.results[e]["y"][:len(idx)].astype(np.float32)
        out[e * TS:(e + 1) * TS] += res.results[e]["ysh"].astype(np.float32)

    if overflow:
        for e in range(E):
            extra = idx_list[e][C:]
            if len(extra) == 0:
                continue
            wgu = _dequant32(gu_p[e], gu_s[e]) / SW
            wd = _dequant32(d_p[e], d_s[e]) / SW
            h = x[extra] @ wgu
            g, u = h[:, :I], h[:, I:]
            a = (g / (1 + np.exp(-g))) * u
            out[extra] += (a @ wd) * combine[extra, e][:, None]
    return out


# revision 3
# speedup vs baseline: 1.0489x; 1.0281x over previous
"""Trainium2 Bass kernel for the FP4-quantized MoE layer — fp8 DoubleRow edition.

Problem (hardcoded): T=2048 tokens, K=2048 hidden, I=1024 intermediate,
E=8 routed experts (top-2, merged), plus an always-on shared expert.

Strategy (8 NeuronCores, expert-parallel; core e owns routed expert e and a
256-token slice of the shared expert):

  * All matmuls run as fp8(e4m3) MatmulPerfMode.DoubleRow: each instruction
    contracts 2x128 rows at 0.5 cycles per output column.
  * Precision: every operand is a 2-plane fp8 decomposition (hi = fp8(v),
    lo = fp8(v - hi)); products accumulate hi*hi + lo*hi + hi*lo in one PSUM
    group (the lo*lo term is dropped; ~1e-3 relative).  Weights are
    pre-scaled by 32 on the host so they sit in e4m3's normal range
    (avoiding the subnormal swamp below 2^-6); activations are re-scaled to
    a 2x domain on the device (max |2*act| ~ 129 < 240).  The scale factors
    cancel via the silu input scale (1/32) and the final copy scale
    (probs/64), both free.
  * Weight planes are decoded+scaled host-side and shipped as fp8 chunk-pair
    tiles; no on-device dequant at all.
  * Outputs ship as fp16 (0.05% of the error budget); host does the
    scatter-add combine.
"""

import numpy as np
import ml_dtypes

import concourse.bacc as bacc
import concourse.bass as bass
import concourse.mybir as mybir
import concourse.tile as tile
from concourse import bass_utils, library_config

F32 = mybir.dt.float32
F16 = mybir.dt.float16
BF16 = mybir.dt.bfloat16
FP8 = mybir.dt.float8e4

NP_F8 = ml_dtypes.float8_e4m3
NP_BF = ml_dtypes.bfloat16

T, K, I, E, TOPK, GS = 2048, 2048, 1024, 8, 2, 64
N_CORES = 8
C = 512            # routed token capacity per expert
TS = T // N_CORES  # shared-expert tokens per core

KP = K // 256      # gate_up contraction chunk-pairs (8)
IP = I // 256      # down contraction chunk-pairs (4)
SW = 32.0          # weight plane domain scale
KA = 2.0           # act plane domain scale
DR = mybir.MatmulPerfMode.DoubleRow

FP4_T = np.array([0, .5, 1, 1.5, 2, 3, 4, 6, 0, -.5, -1, -1.5, -2, -3, -4, -6],
                 dtype=np.float32)

_COMPILED = {}


def _build_program(reps=1):
    nc = bacc.Bacc("TRN2", target_bir_lowering=False, debug=False,
                   num_devices=N_CORES)

    # ---- DRAM I/O (all fp8 planes host-prepared; layouts match SBUF) ----
    # x planes per pair j: [128, 2, tcnt]  (row p -> k = 256j + 128c + p)
    d_xh = nc.dram_tensor("xh", [KP, 128, 2 * C], FP8, kind="ExternalInput")
    d_xl = nc.dram_tensor("xl", [KP, 128, 2 * C], FP8, kind="ExternalInput")
    d_xs = nc.dram_tensor("xs", [KP, 128, 4 * TS], FP8, kind="ExternalInput")
    # gate_up weight planes: per pair j: [128, 2, 2I]  (cols = c*2I + n)
    d_wguh = nc.dram_tensor("wguh", [KP, 128, 2 * 2 * I], FP8,
                            kind="ExternalInput")
    d_wgul = nc.dram_tensor("wgul", [KP, 128, 2 * 2 * I], FP8,
                            kind="ExternalInput")
    d_wdh = nc.dram_tensor("wdh", [IP, 128, 2 * K], FP8, kind="ExternalInput")
    d_wdl = nc.dram_tensor("wdl", [IP, 128, 2 * K], FP8, kind="ExternalInput")
    d_wsguh = nc.dram_tensor("wsguh", [KP, 128, 2 * 2 * I], FP8,
                             kind="ExternalInput")
    d_wsgul = nc.dram_tensor("wsgul", [KP, 128, 2 * 2 * I], FP8,
                             kind="ExternalInput")
    d_wsdh = nc.dram_tensor("wsdh", [IP, 128, 2 * K], FP8, kind="ExternalInput")
    d_wsdl = nc.dram_tensor("wsdl", [IP, 128, 2 * K], FP8, kind="ExternalInput")
    d_pr = nc.dram_tensor("pr", [128, C // 128], F32, kind="ExternalInput")
    d_y = nc.dram_tensor("y", [C, K], F16, kind="ExternalOutput")
    d_ysh = nc.dram_tensor("ysh", [TS, K], F16, kind="ExternalOutput")

    with tile.TileContext(nc) as tc:
        with (
            tc.tile_pool(name="wgu", bufs=3 * KP - 2) as wgu_pool,
            tc.tile_pool(name="wd", bufs=2 * IP + 2) as wd_pool,
            tc.tile_pool(name="xp", bufs=2 * KP) as x_pool,
            tc.tile_pool(name="xsp", bufs=KP) as xs_pool,
            tc.tile_pool(name="pr", bufs=1) as pr_pool,
            tc.tile_pool(name="sil", bufs=2) as sil_pool,
            tc.tile_pool(name="a32", bufs=2) as a32_pool,
            tc.tile_pool(name="ak", bufs=2) as ak_pool,
            tc.tile_pool(name="ah", bufs=4) as ah_pool,
            tc.tile_pool(name="al", bufs=4) as al_pool,
            tc.tile_pool(name="ysb", bufs=22) as ysb_pool,
            tc.tile_pool(name="ps", bufs=8, space="PSUM") as ps_pool,
        ):
            nc.gpsimd.load_library(library_config.standard)

            for _rep in range(reps):
                # ---- input DMAs, in transfer-priority order (all on SP) ----
                def wload(pool, dram, n, width, tag):
                    ts = []
                    for j in range(n):
                        wt = pool.tile([128, width], FP8, tag=tag, name="wt")
                        nc.sync.dma_start(wt[:], dram[j, :, :])
                        ts.append(wt)
                    return ts

                # cold start: interleave x pairs with gate_up hi pairs so the
                # first PSUM groups can open as soon as possible
                xh_t, wguh_t = [], []
                pr_t = None
                for j in range(KP):
                    xt = x_pool.tile([128, 2 * C], FP8, tag="xh", name="xt")
                    nc.sync.dma_start(xt[:], d_xh[j, :, :])
                    xh_t.append(xt)
                    wt = wgu_pool.tile([128, 2 * 2 * I], FP8, tag="wgu",
                                       name="wt")
                    nc.sync.dma_start(wt[:], d_wguh[j, :, :])
                    wguh_t.append(wt)
                    if j == 0:
                        pr_t = pr_pool.tile([128, C // 128], F32, tag="pr")
                        nc.sync.dma_start(pr_t[:], d_pr[:, :])
                xl_t = wload(x_pool, d_xl, KP, 2 * C, "xl")
                wgul_t = wload(wgu_pool, d_wgul, KP, 2 * 2 * I, "wgu")
                wdh_t = wload(wd_pool, d_wdh, IP, 2 * K, "wd")
                wdl_t = wload(wd_pool, d_wdl, IP, 2 * K, "wd")
                xs_t = wload(xs_pool, d_xs, KP, 4 * TS, "xs")
                wsguh_t = wload(wgu_pool, d_wsguh, KP, 2 * 2 * I, "wgu")
                wsgul_t = wload(wgu_pool, d_wsgul, KP, 2 * 2 * I, "wgu")
                wsdh_t = wload(wd_pool, d_wsdh, IP, 2 * K, "wd")
                wsdl_t = wload(wd_pool, d_wsdl, IP, 2 * K, "wd")

                def gu_phase(whi, wlo, xhs, xls, tcnt, ah_ts, al_ts):
                    """gate_up matmuls (DoubleRow, 3 pass-sets) + act planes.

                    k-pair-outer wave order so the PE consumes each weight
                    pair across every open PSUM group as it arrives from DMA
                    (per-bank order would serialize on pair arrivals).
                    Routed (tcnt=512): gate/up each own a bank; waves of 4
                    chunks.  Shared (tcnt=256): gate|up packed into one bank;
                    a single wave of all 8 chunks.
                    """
                    nbt = tcnt // 256
                    packed = nbt == 1
                    x3h, x3l = [], []
                    for th, tl in zip(xhs, xls):
                        x3h.append(th.rearrange("p (c t) -> p c t", c=2))
                        x3l.append(tl.rearrange("p (c t) -> p c t", c=2))
                    waves = ([list(range(8))] if packed
                             else [[0, 1], [2, 3], [4, 5], [6, 7]])
                    for wave in waves:
                        ps_full = {}
                        ps_of = {}
                        for c in wave:
                            if packed:
                                ps = ps_pool.tile([128, 512], F32, tag="ps",
                                                  name="pst")
                                ps_full[c] = ps
                                ps_of[(c, 0)] = ps[:, 0:256]
                                ps_of[(c, 1)] = ps[:, 256:512]
                            else:
                                for half in range(2):
                                    ps = ps_pool.tile([128, 512], F32,
                                                      tag="ps", name="pst")
                                    ps_full[(c, half)] = ps
                                    ps_of[(c, half)] = ps
                        started = set()
                        passes = ((x3h, whi), (x3l, whi), (x3h, wlo))
                        for pi, (xt3, wts) in enumerate(passes):
                            # first two passes j-outer (stream weight-pair
                            # arrivals across all groups); final pass c-outer
                            # so each chunk's group closes early and its act
                            # chain overlaps the remaining matmuls
                            if pi < 2:
                                order = [(j, c) for j in range(KP)
                                         for c in wave]
                            else:
                                order = [(j, c) for c in wave
                                         for j in range(KP)]
                            for j, c in order:
                                if True:
                                    for half in range(2):
                                        w3 = wts[j].rearrange(
                                            "p (c n) -> p c n", c=2)
                                        lhsT = w3[:, :,
                                                  half * I + c * 128:
                                                  half * I + (c + 1) * 128]
                                        gkey = c if packed else (c, half)
                                        for tb in range(nbt):
                                            rhs = xt3[j][:, :,
                                                         tb * 256:
                                                         (tb + 1) * 256]
                                            out = ps_of[(c, half)]
                                            if not packed:
                                                out = out[:, tb * 256:
                                                          (tb + 1) * 256]
                                            last = (pi == 2 and j == KP - 1
                                                    and half == 1
                                                    and tb == nbt - 1)
                                            nc.tensor.matmul(
                                                out, lhsT, rhs,
                                                start=gkey not in started,
                                                stop=last, perf_mode=DR)
                                            started.add(gkey)
                        for c in wave:
                            if packed:
                                gate_ps = ps_full[c][:, 0:256]
                                up_ps = ps_full[c][:, 256:512]
                            else:
                                gate_ps = ps_full[(c, 0)][:, 0:tcnt]
                                up_ps = ps_full[(c, 1)][:, 0:tcnt]
                            sil = sil_pool.tile([128, 512], BF16, tag="sil",
                                                name="silt")
                            nc.scalar.activation(
                                sil[:, 0:tcnt], gate_ps,
                                mybir.ActivationFunctionType.Silu,
                                scale=1.0 / SW)
                            a32 = a32_pool.tile([128, 512], BF16, tag="a32",
                                                name="a32t")
                            nc.vector.tensor_tensor(
                                a32[:, 0:tcnt], sil[:, 0:tcnt], up_ps,
                                mybir.AluOpType.mult)
                            ak = ak_pool.tile([128, 512], BF16, tag="ak",
                                              name="akt")
                            nc.vector.tensor_scalar_mul(
                                ak[:, 0:tcnt], a32[:, 0:tcnt], KA / SW)
                            p, slot = c // 2, c % 2
                            nc.vector.tensor_copy(
                                ah_ts[p][:, slot * 512:slot * 512 + tcnt],
                                ak[:, 0:tcnt])
                            nc.vector.tensor_tensor(
                                al_ts[p][:, slot * 512:slot * 512 + tcnt],
                                ak[:, 0:tcnt],
                                ah_ts[p][:, slot * 512:slot * 512 + tcnt],
                                mybir.AluOpType.subtract)

                def d_phase(wdhi, wdlo, ah_ts, al_ts, tcnt, y_dram, pr_ap):
                    """down matmuls (DoubleRow, 3 pass-sets) + scaled output."""
                    for tb2 in range(tcnt // 128):
                        for kq in range(K // 512):
                            ps = ps_pool.tile([128, 512], F32, tag="ps", name="pst")
                            first = True
                            for kh in range(2):
                                for p in range(IP):
                                    for ats, wts in ((ah_ts, wdhi),
                                                     (al_ts, wdhi),
                                                     (ah_ts, wdlo)):
                                        a3 = ats[p].rearrange(
                                            "p (c t) -> p c t", c=2)
                                        lhsT = a3[:, :,
                                                  tb2 * 128:(tb2 + 1) * 128]
                                        w3 = wts[p].rearrange(
                                            "p (c k) -> p c k", c=2)
                                        rhs = w3[:, :,
                                                 kq * 512 + kh * 256:
                                                 kq * 512 + (kh + 1) * 256]
                                        last = (kh == 1 and p == IP - 1
                                                and wts is wdlo)
                                        nc.tensor.matmul(
                                            ps[:, kh * 256:(kh + 1) * 256],
                                            lhsT, rhs,
                                            start=first, stop=last,
                                            perf_mode=DR)
                                        first = False
                            ot = ysb_pool.tile([128, 512], F16, tag="ysb", name="ysbt")
                            last_it = (pr_ap is None
                                       and tb2 == tcnt // 128 - 1
                                       and kq == K // 512 - 1)
                            if last_it:
                                # kernel tail: split copy across ACT/DVE and
                                # the DMA across SWDGE/HWDGE paths
                                nc.vector.tensor_scalar_mul(
                                    ot[:, 0:256], ps[:, 0:256],
                                    1.0 / (SW * KA))
                                nc.scalar.activation(
                                    ot[:, 256:512], ps[:, 256:512],
                                    mybir.ActivationFunctionType.Copy,
                                    scale=1.0 / (SW * KA))
                                nc.sync.dma_start(
                                    y_dram[tb2 * 128:(tb2 + 1) * 128,
                                           kq * 512:kq * 512 + 256],
                                    ot[:, 0:256])
                                nc.scalar.dma_start(
                                    y_dram[tb2 * 128:(tb2 + 1) * 128,
                                           kq * 512 + 256:(kq + 1) * 512],
                                    ot[:, 256:512])
                            elif pr_ap is None:
                                nc.scalar.activation(
                                    ot[:], ps[:],
                                    mybir.ActivationFunctionType.Copy,
                                    scale=1.0 / (SW * KA))
                                eng = (nc.gpsimd, nc.scalar,
                                       nc.sync)[(tb2 * (K // 512) + kq) % 3]
                                eng.dma_start(
                                    y_dram[tb2 * 128:(tb2 + 1) * 128,
                                           kq * 512:(kq + 1) * 512], ot[:])
                            else:
                                nc.scalar.activation(
                                    ot[:], ps[:],
                                    mybir.ActivationFunctionType.Copy,
                                    scale=pr_ap[:, tb2:tb2 + 1])
                                nc.gpsimd.dma_start(
                                    y_dram[tb2 * 128:(tb2 + 1) * 128,
                                           kq * 512:(kq + 1) * 512], ot[:])

                # act plane tiles (pair tiles: [slot0 | slot1], 512 cols each)
                r_ah = [ah_pool.tile([128, 1024], FP8, tag="ah", name="aht")
                        for _ in range(IP)]
                r_al = [al_pool.tile([128, 1024], FP8, tag="al", name="alt")
                        for _ in range(IP)]
                gu_phase(wguh_t, wgul_t, xh_t, xl_t, C, r_ah, r_al)
                d_phase(wdh_t, wdl_t, r_ah, r_al, C, d_y, pr_t)

                s_ah = [ah_pool.tile([128, 1024], FP8, tag="ah", name="aht")
                        for _ in range(IP)]
                s_al = [al_pool.tile([128, 1024], FP8, tag="al", name="alt")
                        for _ in range(IP)]
                gu_phase(wsguh_t, wsgul_t,
                         [t[:, 0:2 * TS] for t in xs_t],
                         [t[:, 2 * TS:4 * TS] for t in xs_t],
                         TS, s_ah, s_al)
                d_phase(wsdh_t, wsdl_t, s_ah, s_al, TS, d_ysh, None)

    nc.compile()
    return nc


def _get_program():
    if "nc" not in _COMPILED:
        _COMPILED["nc"] = _build_program()
    return _COMPILED["nc"]


def _dequant32(packed, scales):
    """fp4-packed [R/8, N] + scales [R/GS, N] -> 32x-scaled fp32 [R, N]."""
    shifts = (np.arange(8, dtype=np.int32) * 4)[None, :, None]
    nib = (packed[:, None, :] >> shifts) & 0xF
    w = FP4_T[nib].reshape(packed.shape[0] * 8, packed.shape[1])
    return (w * np.repeat(scales.astype(np.float32), GS, axis=0)) * SW


def _planes(w32):
    """fp32 (already 32x) -> (hi, lo) fp8 planes."""
    hi = w32.astype(NP_F8)
    lo = (w32 - hi.astype(np.float32)).astype(NP_F8)
    return hi, lo


def _pair_layout_w(plane, npair):
    """[R, N] -> [npair, 128, 2*N]: row r = 256j + 128c + p."""
    R, N = plane.shape
    out = plane.reshape(npair, 2, 128, N).transpose(0, 2, 1, 3)
    return np.ascontiguousarray(out.reshape(npair, 128, 2 * N))


def _pair_layout_x(xT):
    """[K, tcnt] -> [KP, 128, 2*tcnt] chunk-pair layout."""
    Kd, tc = xT.shape
    out = xT.reshape(KP, 2, 128, tc).transpose(0, 2, 1, 3)
    return np.ascontiguousarray(out.reshape(KP, 128, 2 * tc))


def kernel(**inputs) -> np.ndarray:
    x = np.asarray(inputs["hidden_states"], np.float32)          # [T, K]
    gu_p = np.asarray(inputs["gate_up_weight_packed"])           # [E, K/8, 2I]
    gu_s = np.asarray(inputs["gate_up_scales"], np.float32)
    d_p = np.asarray(inputs["down_weight_packed"])               # [E, I/8, K]
    d_s = np.asarray(inputs["down_scales"], np.float32)
    sgu_p = np.asarray(inputs["shared_gate_up_packed"])
    sgu_s = np.asarray(inputs["shared_gate_up_scales"], np.float32)
    sd_p = np.asarray(inputs["shared_down_packed"])
    sd_s = np.asarray(inputs["shared_down_scales"], np.float32)
    eids = np.asarray(inputs["expert_ids"])
    eprobs = np.asarray(inputs["expert_probs"], np.float32)

    # ---- host routing ----
    combine = np.zeros((T, E), np.float32)
    np.add.at(combine, (np.arange(T)[:, None], eids), eprobs)
    idx_list = [np.nonzero(combine[:, e])[0] for e in range(E)]
    overflow = max(len(i) for i in idx_list) > C

    # x planes, full [K, T] once
    xT = np.ascontiguousarray(x.T)
    xh_full = xT.astype(NP_F8)
    xl_full = (xT - xh_full.astype(np.float32)).astype(NP_F8)

    # shared weight planes (identical on every core)
    wsgu_hi, wsgu_lo = _planes(_dequant32(sgu_p, sgu_s))
    wsd_hi, wsd_lo = _planes(_dequant32(sd_p, sd_s))
    shared_w = {
        "wsguh": _pair_layout_w(wsgu_hi, KP),
        "wsgul": _pair_layout_w(wsgu_lo, KP),
        "wsdh": _pair_layout_w(wsd_hi, IP),
        "wsdl": _pair_layout_w(wsd_lo, IP),
    }

    in_maps = []
    for e in range(E):
        idx = idx_list[e][:C]
        ncnt = len(idx)
        xh_e = np.zeros((K, C), NP_F8)
        xl_e = np.zeros((K, C), NP_F8)
        xh_e[:, :ncnt] = xh_full[:, idx]
        xl_e[:, :ncnt] = xl_full[:, idx]
        pr_full = np.zeros(C, np.float32)
        pr_full[:ncnt] = combine[idx, e] / (SW * KA)
        pr_e = np.ascontiguousarray(pr_full.reshape(C // 128, 128).T)

        wgu_hi, wgu_lo = _planes(_dequant32(gu_p[e], gu_s[e]))
        wd_hi, wd_lo = _planes(_dequant32(d_p[e], d_s[e]))
        sl = slice(e * TS, (e + 1) * TS)
        in_maps.append({
            "xh": _pair_layout_x(xh_e),
            "xl": _pair_layout_x(xl_e),
            "xs": np.concatenate([_pair_layout_x(xh_full[:, sl]),
                                  _pair_layout_x(xl_full[:, sl])], axis=2),
            "wguh": _pair_layout_w(wgu_hi, KP),
            "wgul": _pair_layout_w(wgu_lo, KP),
            "wdh": _pair_layout_w(wd_hi, IP),
            "wdl": _pair_layout_w(wd_lo, IP),
            "pr": pr_e,
            **shared_w,
        })

    nc = _get_program()
    res = bass_utils.run_bass_kernel_spmd(nc, in_maps,
                                          core_ids=list(range(N_CORES)))

    # ---- host combine ----
    out = np.zeros((T, K), np.float32)
    for e in range(E):
        idx = idx_list[e][:C]
        out[idx] += res.results[e]["y"][:len(idx)].astype(np.float32)
        out[e * TS:(e + 1) * TS] += res.results[e]["ysh"].astype(np.float32)

    if overflow:
        for e in range(E):
            extra = idx_list[e][C:]
            if len(extra) == 0:
                continue
            wgu = _dequant32(gu_p[e], gu_s[e]) / SW
            wd = _dequant32(d_p[e], d_s[e]) / SW
            h = x[extra] @ wgu
            g, u = h[:, :I], h[:, I:]
            a = (g / (1 + np.exp(-g))) * u
            out[extra] += (a @ wd) * combine[extra, e][:, None]
    return out


# revision 4
# speedup vs baseline: 1.0578x; 1.0084x over previous
"""Trainium2 Bass kernel for the FP4-quantized MoE layer — fp8 DoubleRow edition.

Problem (hardcoded): T=2048 tokens, K=2048 hidden, I=1024 intermediate,
E=8 routed experts (top-2, merged), plus an always-on shared expert.

Strategy (8 NeuronCores, expert-parallel; core e owns routed expert e and a
256-token slice of the shared expert):

  * All matmuls run as fp8(e4m3) MatmulPerfMode.DoubleRow: each instruction
    contracts 2x128 rows at 0.5 cycles per output column.
  * Precision: every operand is a 2-plane fp8 decomposition (hi = fp8(v),
    lo = fp8(v - hi)); products accumulate hi*hi + lo*hi + hi*lo in one PSUM
    group (the lo*lo term is dropped; ~1e-3 relative).  Weights are
    pre-scaled by 32 on the host so they sit in e4m3's normal range
    (avoiding the subnormal swamp below 2^-6); activations are re-scaled to
    a 2x domain on the device (max |2*act| ~ 129 < 240).  The scale factors
    cancel via the silu input scale (1/32) and the final copy scale
    (probs/64), both free.
  * Weight planes are decoded+scaled host-side and shipped as fp8 chunk-pair
    tiles; no on-device dequant at all.
  * Outputs ship as fp16 (0.05% of the error budget); host does the
    scatter-add combine.
"""

import numpy as np
import ml_dtypes

import concourse.bacc as bacc
import concourse.bass as bass
import concourse.mybir as mybir
import concourse.tile as tile
from concourse import bass_utils, library_config

F32 = mybir.dt.float32
F16 = mybir.dt.float16
BF16 = mybir.dt.bfloat16
FP8 = mybir.dt.float8e4

NP_F8 = ml_dtypes.float8_e4m3
NP_BF = ml_dtypes.bfloat16

T, K, I, E, TOPK, GS = 2048, 2048, 1024, 8, 2, 64
N_CORES = 8
C = 512            # routed token capacity per expert
TS = T // N_CORES  # shared-expert tokens per core

KP = K // 256      # gate_up contraction chunk-pairs (8)
IP = I // 256      # down contraction chunk-pairs (4)
SW = 32.0          # weight plane domain scale
KA = 2.0           # act plane domain scale
DR = mybir.MatmulPerfMode.DoubleRow

FP4_T = np.array([0, .5, 1, 1.5, 2, 3, 4, 6, 0, -.5, -1, -1.5, -2, -3, -4, -6],
                 dtype=np.float32)

_COMPILED = {}


def _build_program(reps=1):
    nc = bacc.Bacc("TRN2", target_bir_lowering=False, debug=False,
                   num_devices=N_CORES)

    # ---- DRAM I/O (all fp8 planes host-prepared; layouts match SBUF) ----
    # x planes per pair j: [128, 2, tcnt]  (row p -> k = 256j + 128c + p)
    d_xh = nc.dram_tensor("xh", [KP, 128, 2 * C], FP8, kind="ExternalInput")
    d_xl = nc.dram_tensor("xl", [KP, 128, 2 * C], FP8, kind="ExternalInput")
    d_xs = nc.dram_tensor("xs", [KP, 128, 4 * TS], FP8, kind="ExternalInput")
    # gate_up weight planes: per pair j: [128, 2, 2I]  (cols = c*2I + n)
    d_wguh = nc.dram_tensor("wguh", [KP, 128, 2 * 2 * I], FP8,
                            kind="ExternalInput")
    d_wgul = nc.dram_tensor("wgul", [KP, 128, 2 * 2 * I], FP8,
                            kind="ExternalInput")
    d_wdh = nc.dram_tensor("wdh", [IP, 128, 2 * K], FP8, kind="ExternalInput")
    d_wdl = nc.dram_tensor("wdl", [IP, 128, 2 * K], FP8, kind="ExternalInput")
    d_wsguh = nc.dram_tensor("wsguh", [KP, 128, 2 * 2 * I], FP8,
                             kind="ExternalInput")
    d_wsgul = nc.dram_tensor("wsgul", [KP, 128, 2 * 2 * I], FP8,
                             kind="ExternalInput")
    d_wsdh = nc.dram_tensor("wsdh", [IP, 128, 2 * K], FP8, kind="ExternalInput")
    d_wsdl = nc.dram_tensor("wsdl", [IP, 128, 2 * K], FP8, kind="ExternalInput")
    d_pr = nc.dram_tensor("pr", [128, C // 128], F32, kind="ExternalInput")
    d_y = nc.dram_tensor("y", [C, K], F16, kind="ExternalOutput")
    d_ysh = nc.dram_tensor("ysh", [TS, K], F16, kind="ExternalOutput")

    with tile.TileContext(nc) as tc:
        with (
            tc.tile_pool(name="wgu", bufs=3 * KP - 2) as wgu_pool,
            tc.tile_pool(name="wd", bufs=2 * IP + 2) as wd_pool,
            tc.tile_pool(name="xp", bufs=2 * KP) as x_pool,
            tc.tile_pool(name="xsp", bufs=KP) as xs_pool,
            tc.tile_pool(name="pr", bufs=1) as pr_pool,
            tc.tile_pool(name="sil", bufs=3) as sil_pool,
            tc.tile_pool(name="a32", bufs=3) as a32_pool,
            tc.tile_pool(name="ak", bufs=2) as ak_pool,
            tc.tile_pool(name="ah", bufs=4) as ah_pool,
            tc.tile_pool(name="al", bufs=4) as al_pool,
            tc.tile_pool(name="ysb", bufs=20) as ysb_pool,
            tc.tile_pool(name="ps", bufs=8, space="PSUM") as ps_pool,
        ):
            nc.gpsimd.load_library(library_config.standard)

            for _rep in range(reps):
                # ---- input DMAs, in transfer-priority order (all on SP) ----
                def wload(pool, dram, n, width, tag):
                    ts = []
                    for j in range(n):
                        wt = pool.tile([128, width], FP8, tag=tag, name="wt")
                        nc.sync.dma_start(wt[:], dram[j, :, :])
                        ts.append(wt)
                    return ts

                # cold start: interleave x pairs with gate_up hi pairs so the
                # first PSUM groups can open as soon as possible
                xh_t, wguh_t = [], []
                pr_t = None
                for j in range(KP):
                    xt = x_pool.tile([128, 2 * C], FP8, tag="xh", name="xt")
                    nc.sync.dma_start(xt[:], d_xh[j, :, :])
                    xh_t.append(xt)
                    wt = wgu_pool.tile([128, 2 * 2 * I], FP8, tag="wgu",
                                       name="wt")
                    nc.sync.dma_start(wt[:], d_wguh[j, :, :])
                    wguh_t.append(wt)
                    if j == 0:
                        pr_t = pr_pool.tile([128, C // 128], F32, tag="pr")
                        nc.sync.dma_start(pr_t[:], d_pr[:, :])
                xl_t = wload(x_pool, d_xl, KP, 2 * C, "xl")
                wgul_t = wload(wgu_pool, d_wgul, KP, 2 * 2 * I, "wgu")
                wdh_t = wload(wd_pool, d_wdh, IP, 2 * K, "wd")
                wdl_t = wload(wd_pool, d_wdl, IP, 2 * K, "wd")
                xs_t = wload(xs_pool, d_xs, KP, 4 * TS, "xs")
                wsguh_t = wload(wgu_pool, d_wsguh, KP, 2 * 2 * I, "wgu")
                wsgul_t = wload(wgu_pool, d_wsgul, KP, 2 * 2 * I, "wgu")
                wsdh_t = wload(wd_pool, d_wsdh, IP, 2 * K, "wd")
                wsdl_t = wload(wd_pool, d_wsdl, IP, 2 * K, "wd")

                def gu_phase(whi, wlo, xhs, xls, tcnt, ah_ts, al_ts):
                    """gate_up matmuls (DoubleRow, 3 pass-sets) + act planes.

                    k-pair-outer wave order so the PE consumes each weight
                    pair across every open PSUM group as it arrives from DMA
                    (per-bank order would serialize on pair arrivals).
                    Routed (tcnt=512): gate/up each own a bank; waves of 4
                    chunks.  Shared (tcnt=256): gate|up packed into one bank;
                    a single wave of all 8 chunks.
                    """
                    nbt = tcnt // 256
                    packed = nbt == 1
                    x3h, x3l = [], []
                    for th, tl in zip(xhs, xls):
                        x3h.append(th.rearrange("p (c t) -> p c t", c=2))
                        x3l.append(tl.rearrange("p (c t) -> p c t", c=2))
                    waves = ([list(range(8))] if packed
                             else [[0, 1], [2, 3], [4, 5], [6, 7]])
                    for wave in waves:
                        ps_full = {}
                        ps_of = {}
                        for c in wave:
                            if packed:
                                ps = ps_pool.tile([128, 512], F32, tag="ps",
                                                  name="pst")
                                ps_full[c] = ps
                                ps_of[(c, 0)] = ps[:, 0:256]
                                ps_of[(c, 1)] = ps[:, 256:512]
                            else:
                                for half in range(2):
                                    ps = ps_pool.tile([128, 512], F32,
                                                      tag="ps", name="pst")
                                    ps_full[(c, half)] = ps
                                    ps_of[(c, half)] = ps
                        started = set()
                        passes = ((x3h, whi), (x3l, whi), (x3h, wlo))
                        for pi, (xt3, wts) in enumerate(passes):
                            # first two passes j-outer (stream weight-pair
                            # arrivals across all groups); final pass c-outer
                            # so each chunk's group closes early and its act
                            # chain overlaps the remaining matmuls
                            if pi < 2:
                                order = [(j, c) for j in range(KP)
                                         for c in wave]
                            else:
                                order = [(j, c) for c in wave
                                         for j in range(KP)]
                            for j, c in order:
                                if True:
                                    for half in range(2):
                                        w3 = wts[j].rearrange(
                                            "p (c n) -> p c n", c=2)
                                        lhsT = w3[:, :,
                                                  half * I + c * 128:
                                                  half * I + (c + 1) * 128]
                                        gkey = c if packed else (c, half)
                                        for tb in range(nbt):
                                            rhs = xt3[j][:, :,
                                                         tb * 256:
                                                         (tb + 1) * 256]
                                            out = ps_of[(c, half)]
                                            if not packed:
                                                out = out[:, tb * 256:
                                                          (tb + 1) * 256]
                                            last = (pi == 2 and j == KP - 1
                                                    and half == 1
                                                    and tb == nbt - 1)
                                            nc.tensor.matmul(
                                                out, lhsT, rhs,
                                                start=gkey not in started,
                                                stop=last, perf_mode=DR)
                                            started.add(gkey)
                        for c in wave:
                            if packed:
                                gate_ps = ps_full[c][:, 0:256]
                                up_ps = ps_full[c][:, 256:512]
                            else:
                                gate_ps = ps_full[(c, 0)][:, 0:tcnt]
                                up_ps = ps_full[(c, 1)][:, 0:tcnt]
                            sil = sil_pool.tile([128, 512], BF16, tag="sil",
                                                name="silt")
                            nc.scalar.activation(
                                sil[:, 0:tcnt], gate_ps,
                                mybir.ActivationFunctionType.Silu,
                                scale=1.0 / SW)
                            a32 = a32_pool.tile([128, 512], BF16, tag="a32",
                                                name="a32t")
                            nc.vector.tensor_tensor(
                                a32[:, 0:tcnt], sil[:, 0:tcnt], up_ps,
                                mybir.AluOpType.mult)
                            ak = ak_pool.tile([128, 512], BF16, tag="ak",
                                              name="akt")
                            nc.vector.tensor_scalar_mul(
                                ak[:, 0:tcnt], a32[:, 0:tcnt], KA / SW)
                            p, slot = c // 2, c % 2
                            nc.vector.tensor_copy(
                                ah_ts[p][:, slot * 512:slot * 512 + tcnt],
                                ak[:, 0:tcnt])
                            nc.vector.tensor_tensor(
                                al_ts[p][:, slot * 512:slot * 512 + tcnt],
                                ak[:, 0:tcnt],
                                ah_ts[p][:, slot * 512:slot * 512 + tcnt],
                                mybir.AluOpType.subtract)

                def d_phase(wdhi, wdlo, ah_ts, al_ts, tcnt, y_dram, pr_ap):
                    """down matmuls (DoubleRow, 3 pass-sets) + scaled output."""
                    for tb2 in range(tcnt // 128):
                        for kq in range(K // 512):
                            final_it = (pr_ap is None
                                        and tb2 == tcnt // 128 - 1
                                        and kq == K // 512 - 1)
                            if final_it:
                                # two half-width groups in separate banks so
                                # the very last output chain is 256 cols
                                psh = [ps_pool.tile([128, 512], F32, tag="ps",
                                                    name="pst")
                                       for _ in range(2)]
                            else:
                                ps = ps_pool.tile([128, 512], F32, tag="ps",
                                                  name="pst")
                            first = [True, True]
                            for kh in range(2):
                                for p in range(IP):
                                    for ats, wts in ((ah_ts, wdhi),
                                                     (al_ts, wdhi),
                                                     (ah_ts, wdlo)):
                                        a3 = ats[p].rearrange(
                                            "p (c t) -> p c t", c=2)
                                        lhsT = a3[:, :,
                                                  tb2 * 128:(tb2 + 1) * 128]
                                        w3 = wts[p].rearrange(
                                            "p (c k) -> p c k", c=2)
                                        rhs = w3[:, :,
                                                 kq * 512 + kh * 256:
                                                 kq * 512 + (kh + 1) * 256]
                                        last = (p == IP - 1 and wts is wdlo
                                                if final_it
                                                else (kh == 1 and p == IP - 1
                                                      and wts is wdlo))
                                        out = (psh[kh][:, 0:256] if final_it
                                               else ps[:, kh * 256:
                                                       (kh + 1) * 256])
                                        nc.tensor.matmul(
                                            out, lhsT, rhs,
                                            start=first[kh if final_it else 0],
                                            stop=last, perf_mode=DR)
                                        first[kh if final_it else 0] = False
                            ot = ysb_pool.tile([128, 512], F16, tag="ysb", name="ysbt")
                            last_it = (pr_ap is None
                                       and tb2 == tcnt // 128 - 1
                                       and kq == K // 512 - 1)
                            if last_it:
                                # kernel tail: split copy across ACT/DVE and
                                # the DMA across SWDGE/HWDGE paths
                                nc.vector.tensor_scalar_mul(
                                    ot[:, 0:256], psh[0][:, 0:256],
                                    1.0 / (SW * KA))
                                nc.scalar.activation(
                                    ot[:, 256:512], psh[1][:, 0:256],
                                    mybir.ActivationFunctionType.Copy,
                                    scale=1.0 / (SW * KA))
                                nc.sync.dma_start(
                                    y_dram[tb2 * 128:(tb2 + 1) * 128,
                                           kq * 512:kq * 512 + 256],
                                    ot[:, 0:256])
                                nc.scalar.dma_start(
                                    y_dram[tb2 * 128:(tb2 + 1) * 128,
                                           kq * 512 + 256:(kq + 1) * 512],
                                    ot[:, 256:512])
                            elif pr_ap is None:
                                nc.scalar.activation(
                                    ot[:], ps[:],
                                    mybir.ActivationFunctionType.Copy,
                                    scale=1.0 / (SW * KA))
                                eng = (nc.gpsimd, nc.scalar,
                                       nc.sync)[(tb2 * (K // 512) + kq) % 3]
                                eng.dma_start(
                                    y_dram[tb2 * 128:(tb2 + 1) * 128,
                                           kq * 512:(kq + 1) * 512], ot[:])
                            else:
                                nc.scalar.activation(
                                    ot[:], ps[:],
                                    mybir.ActivationFunctionType.Copy,
                                    scale=pr_ap[:, tb2:tb2 + 1])
                                nc.gpsimd.dma_start(
                                    y_dram[tb2 * 128:(tb2 + 1) * 128,
                                           kq * 512:(kq + 1) * 512], ot[:])

                # act plane tiles (pair tiles: [slot0 | slot1], 512 cols each)
                r_ah = [ah_pool.tile([128, 1024], FP8, tag="ah", name="aht")
                        for _ in range(IP)]
                r_al = [al_pool.tile([128, 1024], FP8, tag="al", name="alt")
                        for _ in range(IP)]
                gu_phase(wguh_t, wgul_t, xh_t, xl_t, C, r_ah, r_al)
                d_phase(wdh_t, wdl_t, r_ah, r_al, C, d_y, pr_t)

                s_ah = [ah_pool.tile([128, 1024], FP8, tag="ah", name="aht")
                        for _ in range(IP)]
                s_al = [al_pool.tile([128, 1024], FP8, tag="al", name="alt")
                        for _ in range(IP)]
                gu_phase(wsguh_t, wsgul_t,
                         [t[:, 0:2 * TS] for t in xs_t],
                         [t[:, 2 * TS:4 * TS] for t in xs_t],
                         TS, s_ah, s_al)
                d_phase(wsdh_t, wsdl_t, s_ah, s_al, TS, d_ysh, None)

    nc.compile()
    return nc


def _get_program():
    if "nc" not in _COMPILED:
        _COMPILED["nc"] = _build_program()
    return _COMPILED["nc"]


def _dequant32(packed, scales):
    """fp4-packed [R/8, N] + scales [R/GS, N] -> 32x-scaled fp32 [R, N]."""
    shifts = (np.arange(8, dtype=np.int32) * 4)[None, :, None]
    nib = (packed[:, None, :] >> shifts) & 0xF
    w = FP4_T[nib].reshape(packed.shape[0] * 8, packed.shape[1])
    return (w * np.repeat(scales.astype(np.float32), GS, axis=0)) * SW


def _planes(w32):
    """fp32 (already 32x) -> (hi, lo) fp8 planes."""
    hi = w32.astype(NP_F8)
    lo = (w32 - hi.astype(np.float32)).astype(NP_F8)
    return hi, lo


def _pair_layout_w(plane, npair):
    """[R, N] -> [npair, 128, 2*N]: row r = 256j + 128c + p."""
    R, N = plane.shape
    out = plane.reshape(npair, 2, 128, N).transpose(0, 2, 1, 3)
    return np.ascontiguousarray(out.reshape(npair, 128, 2 * N))


def _pair_layout_x(xT):
    """[K, tcnt] -> [KP, 128, 2*tcnt] chunk-pair layout."""
    Kd, tc = xT.shape
    out = xT.reshape(KP, 2, 128, tc).transpose(0, 2, 1, 3)
    return np.ascontiguousarray(out.reshape(KP, 128, 2 * tc))


def kernel(**inputs) -> np.ndarray:
    x = np.asarray(inputs["hidden_states"], np.float32)          # [T, K]
    gu_p = np.asarray(inputs["gate_up_weight_packed"])           # [E, K/8, 2I]
    gu_s = np.asarray(inputs["gate_up_scales"], np.float32)
    d_p = np.asarray(inputs["down_weight_packed"])               # [E, I/8, K]
    d_s = np.asarray(inputs["down_scales"], np.float32)
    sgu_p = np.asarray(inputs["shared_gate_up_packed"])
    sgu_s = np.asarray(inputs["shared_gate_up_scales"], np.float32)
    sd_p = np.asarray(inputs["shared_down_packed"])
    sd_s = np.asarray(inputs["shared_down_scales"], np.float32)
    eids = np.asarray(inputs["expert_ids"])
    eprobs = np.asarray(inputs["expert_probs"], np.float32)

    # ---- host routing ----
    combine = np.zeros((T, E), np.float32)
    np.add.at(combine, (np.arange(T)[:, None], eids), eprobs)
    idx_list = [np.nonzero(combine[:, e])[0] for e in range(E)]
    overflow = max(len(i) for i in idx_list) > C

    # x planes, full [K, T] once
    xT = np.ascontiguousarray(x.T)
    xh_full = xT.astype(NP_F8)
    xl_full = (xT - xh_full.astype(np.float32)).astype(NP_F8)

    # shared weight planes (identical on every core)
    wsgu_hi, wsgu_lo = _planes(_dequant32(sgu_p, sgu_s))
    wsd_hi, wsd_lo = _planes(_dequant32(sd_p, sd_s))
    shared_w = {
        "wsguh": _pair_layout_w(wsgu_hi, KP),
        "wsgul": _pair_layout_w(wsgu_lo, KP),
        "wsdh": _pair_layout_w(wsd_hi, IP),
        "wsdl": _pair_layout_w(wsd_lo, IP),
    }

    in_maps = []
    for e in range(E):
        idx = idx_list[e][:C]
        ncnt = len(idx)
        xh_e = np.zeros((K, C), NP_F8)
        xl_e = np.zeros((K, C), NP_F8)
        xh_e[:, :ncnt] = xh_full[:, idx]
        xl_e[:, :ncnt] = xl_full[:, idx]
        pr_full = np.zeros(C, np.float32)
        pr_full[:ncnt] = combine[idx, e] / (SW * KA)
        pr_e = np.ascontiguousarray(pr_full.reshape(C // 128, 128).T)

        wgu_hi, wgu_lo = _planes(_dequant32(gu_p[e], gu_s[e]))
        wd_hi, wd_lo = _planes(_dequant32(d_p[e], d_s[e]))
        sl = slice(e * TS, (e + 1) * TS)
        in_maps.append({
            "xh": _pair_layout_x(xh_e),
            "xl": _pair_layout_x(xl_e),
            "xs": np.concatenate([_pair_layout_x(xh_full[:, sl]),
                                  _pair_layout_x(xl_full[:, sl])], axis=2),
            "wguh": _pair_layout_w(wgu_hi, KP),
            "wgul": _pair_layout_w(wgu_lo, KP),
            "wdh": _pair_layout_w(wd_hi, IP),
            "wdl": _pair_layout_w(wd_lo, IP),
            "pr": pr_e,
            **shared_w,
        })

    nc = _get_program()
    res = bass_utils.run_bass_kernel_spmd(nc, in_maps,
                                          core_ids=list(range(N_CORES)))

    # ---- host combine ----
    out = np.zeros((T, K), np.float32)
    for e in range(E):
        idx = idx_list[e][:C]
        out[idx] += res.results[e]["y"][:len(idx)].astype(np.float32)
        out[e * TS:(e + 1) * TS] += res.results[e]["ysh"].astype(np.float32)

    if overflow:
        for e in range(E):
            extra = idx_list[e][C:]
            if len(extra) == 0:
                continue
            wgu = _dequant32(gu_p[e], gu_s[e]) / SW
            wd = _dequant32(d_p[e], d_s[e]) / SW
            h = x[extra] @ wgu
            g, u = h[:, :I], h[:, I:]
            a = (g / (1 + np.exp(-g))) * u
            out[extra] += (a @ wd) * combine[extra, e][:, None]
    return out
